# revision 41
# baseline (speedup 1.0000x reference)
"""Trainium2 Bass kernel for DenseEquivariantShiftModule.

shift[b,i,c] = ( sum_k pb[b,i,k,c]*ps[b,i,k]
               + (1/A_b) sum_k sum_j u[b,j]*rb[b,i,j,k,c]*rs[b,i,j,k] ) / A_b
where ps = MLP_pw(pointwise_features), rs = MLP_rel(relative_features),
u = ~masked, A_b = sum_j u[b,j].

Sharding: B*N = 1024 "i" rows split across 8 cores (128 rows each, each
core within one batch element).

v2 design (per core, all bf16 matmuls, f32 PSUM accumulate):
  - host compacts the j axis to the unmasked atoms (448 here) and pads to
    JPAD; masked j contribute exact zeros via zeroed rbw rows.
  - per i-row:
      L1: H1T[h,j] = w1^T @ XT         (1 matmul, 448-moving, w1 stationary)
      relu1 on Act engine, pair-batched, PSUM->SBUF bf16
      L2 flipped: H2[j,h] = (H1T chunk)^T @ w2   (4 chunk matmuls)
      relu2 on DVE (some pairs on Act), pair-batched
      mt flipped: M^T[h,kc] += (H2 chunk)^T @ rbw chunk   (4 tiny matmuls,
        12-moving, accumulated in a shared PSUM bank [128, 16 rows, 12])
  - per 16-row group the raw M^T PSUM bank is DMA'd to DRAM; the host
    epilogue applies W3 (sum over h), the b3*rbsum term, the 1/A factors,
    and combines with the pointwise path (O(N) work, like the baseline's
    host-side gather).
  - pointwise MLP runs feature-major on-device once per core
    (host pre-transposes its input), raw scales DMA'd out.
"""
import sys

sys.path.insert(0, "/opt/trn_rl_repo")

import ml_dtypes
import numpy as np

import concourse.bass as bass
import concourse.tile as tile
from concourse import mybir

B, N, F, NB = 2, 512, 128, 4
NCORES = 8
IPC = B * N // NCORES  # i-rows per core
f32 = mybir.dt.float32
bf16 = mybir.dt.bfloat16
f8 = mybir.dt.float8e4

JPAD_DEFAULT = 448  # unmasked atoms for this problem's mask (last 64 masked)
GROUP = 16  # i-rows per mt PSUM bank / output DMA group
# every RELU2_ACT_EVERY-th pair's relu2 runs on Act instead of DVE
RELU2_ACT_EVERY = 8


def _install_tile_patch():
    """walrus in this container accepts only 1 sem wait per CTRL
    instruction; TileContext's tail drain carries one per touched
    processor. Split them across SP NOPs."""
    import re

    import bass_rust
    from concourse.vector_clock import ScopedClock

    def _patched(self, tick_clock, wait_clock):
        gc = tick_clock.global_clock
        vals = eval(re.match(r"VectorClock\((\[.*\])\)", repr(gc)).group(1))
        for i, v in enumerate(vals):
            if v <= 0:
                continue
            sub = [0] * len(vals)
            sub[i] = v
            nop = self.nc.sync.nop(nofuse=True, hint="drain_wait_split")
            wait_clock.add_sem_waits(
                nop.ins, ScopedClock({None: bass_rust.VectorClock(sub)})
            )
        self.nc.sync.drain()
        self.nc.all_engine_barrier()
        assert self.sems is not None
        popped = self.nc._tile_sem_poison_stack.pop()
        assert popped is self._sem_poison
        self.nc.clear_and_free_semaphores(list(self.sems.allocated().values()))
        self.nc.all_engine_barrier()

    tile.TileContext._drain_and_barrier = _patched


def _split_multi_waits(nc):
    """This walrus build accepts a single sem wait per instruction.
    Move extra waits onto same-engine NOPs inserted just before the
    owning instruction."""
    import bass_rust

    n = 0
    for f in nc.m.functions:
        for bb in f.blocks:
            insts = bb.instructions
            i = 0
            while i < len(insts):
                ins = insts[i]
                si = ins.sync_info
                if si is not None and si.on_wait and len(si.on_wait) > 1:
                    waits = list(si.on_wait)
                    updates = list(si.on_update) if si.on_update else []
                    for w in waits[:-1]:
                        nop = mybir.InstNoOp(
                            name=f"I-waitsplit-{n}", ins=[], outs=[]
                        )
                        n += 1
                        nop.engine = ins.engine
                        nop.sync_info = bass_rust.SyncInfo(
                            on_wait=[w], on_update=[]
                        )
                        insts.insert(i, nop)
                        i += 1
                    ins.sync_info = bass_rust.SyncInfo(
                        on_wait=[waits[-1]], on_update=updates
                    )
                i += 1
    return n




def _pointwise(nc, tc, consts, psB_t, ipc, pw, pscout):
    """Pointwise MLP, feature-major (input pre-transposed on host)."""
    pw1s, pw2s, pw3s, pb1s, pb2s, xpts = pw
    Relu = mybir.ActivationFunctionType.Relu
    Copy = mybir.ActivationFunctionType.Copy
    pr = min(ipc, 512)
    nc.tensor.matmul(psB_t[0][:, 0, 0:pr], pw1s[:], xpts[:])
    h1p = consts.tile([128, ipc], bf16)
    nc.scalar.activation(h1p[:], psB_t[0][:, 0, 0:pr], Relu, bias=pb1s[:])
    nc.tensor.matmul(psB_t[0][:, 1, 0:pr], pw2s[:], h1p[:])
    h2p = consts.tile([128, ipc], bf16)
    nc.scalar.activation(h2p[:], psB_t[0][:, 1, 0:pr], Relu, bias=pb2s[:])
    nc.tensor.matmul(psB_t[1][0:NB, 0, 0:pr], pw3s[:], h2p[:])
    pscsb = consts.tile([NB, ipc], f32)
    nc.scalar.activation(pscsb[:], psB_t[1][0:NB, 0, 0:pr], Copy)
    nc.sync.dma_start(out=pscout[:], in_=pscsb[:])

def build_program(jpad=JPAD_DEFAULT, ipc=IPC, split_waits=True):
    _install_tile_patch()
    assert jpad % 64 == 0
    nch = (jpad + 127) // 128  # j chunks (last may be partial)
    ng4 = ipc // 4  # 4-row DMA groups
    ngr = ipc // GROUP  # output groups
    nc = bass.Bass()

    xr4 = nc.dram_tensor("xr4", [ipc // 8, F, 8, jpad], bf16,
                         kind="ExternalInput")
    rbm = nc.dram_tensor("rbm", [128, ipc, nch, 12], bf16, kind="ExternalInput")
    w1 = nc.dram_tensor("w1", [F, 128], bf16, kind="ExternalInput")
    w2 = nc.dram_tensor("w2", [128, 128], bf16, kind="ExternalInput")
    b1 = nc.dram_tensor("b1", [128, 1], f32, kind="ExternalInput")
    xpt = nc.dram_tensor("xpt", [F, ipc], bf16, kind="ExternalInput")
    pw1 = nc.dram_tensor("pw1", [F, 128], bf16, kind="ExternalInput")
    pw2 = nc.dram_tensor("pw2", [128, 128], bf16, kind="ExternalInput")
    pw3 = nc.dram_tensor("pw3", [128, NB], bf16, kind="ExternalInput")
    pb1 = nc.dram_tensor("pb1", [128, 1], f32, kind="ExternalInput")
    pb2 = nc.dram_tensor("pb2", [128, 1], f32, kind="ExternalInput")
    mtout = nc.dram_tensor("mtout", [ngr, 128, GROUP * 12], bf16,
                           kind="ExternalOutput")
    pscout = nc.dram_tensor("pscout", [NB, ipc], f32, kind="ExternalOutput")

    from contextlib import ExitStack

    with tile.TileContext(nc) as tc:
        with ExitStack() as ctx:
            _kernel_body(ctx, tc, jpad, nch, ipc,
                         xr4, rbm, (w1, w2, b1),
                         (xpt, pw1, pw2, pw3, pb1, pb2),
                         mtout, pscout)
    if split_waits:
        _split_multi_waits(nc)
    return nc


def _kernel_body(ctx, tc, jpad, nch, ipc, xr4, rbm, relw, pww, mtout, pscout):
    nc = tc.nc
    w1, w2, b1 = relw
    xpt, pw1, pw2, pw3, pb1, pb2 = pww
    Relu = mybir.ActivationFunctionType.Relu
    npair = ipc // 2
    ng4 = ipc // 4
    ngr = ipc // GROUP
    ctail = jpad - (nch - 1) * 128  # width of last j-chunk (64 or 128)

    Copy = mybir.ActivationFunctionType.Copy
    consts = ctx.enter_context(tc.tile_pool(name="consts", bufs=1))
    xtpool = ctx.enter_context(tc.tile_pool(name="xt", bufs=2))
    h1pool = ctx.enter_context(tc.tile_pool(name="h1", bufs=3))
    h2pool = ctx.enter_context(tc.tile_pool(name="h2", bufs=3))
    mtpool = ctx.enter_context(tc.tile_pool(name="mtc", bufs=2))
    ps_a = ctx.enter_context(tc.tile_pool(name="ps_a", bufs=1, space="PSUM"))
    ps_b = ctx.enter_context(tc.tile_pool(name="ps_b", bufs=1, space="PSUM"))

    # Static PSUM: 4 banks for L1 outs, 4 banks for L2 outs, organized as
    # TWO tiles per stage (one per pair parity).  Dependency tracking is
    # effectively whole-tile for cross-engine hazards, so separate tiles —
    # not just separate banks — are required for consecutive pairs to
    # pipeline without WAR serialization.
    # The mt accumulators live in the unused 64-f32 tails of the psA banks
    # (L1 writes only [0:jpad]); mt of pair p writes the OPPOSITE parity's
    # tile from the one relu1(p) is reading, so they never couple.
    assert jpad + 4 * 12 <= 512, "mt tails must fit beside L1 outputs"
    psA_t = [ps_a.tile([128, 2, 512], f32, tag=f"a{t}", name=f"psA{t}")
             for t in range(2)]
    psB_t = [ps_b.tile([128, 2, 512], f32, tag=f"b{t}", name=f"psB{t}")
             for t in range(2)]

    # constants (tiny, ahead of the first big X transfer on the same queue)
    w1s = consts.tile([128, 128], bf16)
    nc.sync.dma_start(out=w1s[:], in_=w1[:])
    w2s = consts.tile([128, 128], bf16)
    nc.sync.dma_start(out=w2s[:], in_=w2[:])
    b1s = consts.tile([128, 1], f32)
    nc.sync.dma_start(out=b1s[:], in_=b1[:])

    # pointwise constants first on the SWDGE queue (tiny): the pointwise
    # MLP runs during the startup window while the first X transfer lands
    pw1s = consts.tile([128, 128], bf16)
    nc.gpsimd.dma_start(out=pw1s[:], in_=pw1[:])
    pw2s = consts.tile([128, 128], bf16)
    nc.gpsimd.dma_start(out=pw2s[:], in_=pw2[:])
    pw3s = consts.tile([128, NB], bf16)
    nc.gpsimd.dma_start(out=pw3s[:], in_=pw3[:])
    pb1s = consts.tile([128, 1], f32)
    nc.gpsimd.dma_start(out=pb1s[:], in_=pb1[:])
    pb2s = consts.tile([128, 1], f32)
    nc.gpsimd.dma_start(out=pb2s[:], in_=pb2[:])
    xpts = consts.tile([128, ipc], bf16)
    nc.gpsimd.dma_start(out=xpts[:], in_=xpt[:])

    # rbw chunks, resident in SBUF: [jp, i, c, kc]; SWDGE path (gpsimd)
    # so they stream in parallel with the first X transfers
    rb_all = consts.tile([128, ipc, nch, 12], bf16)
    for q in range(8):
        i0, i1 = q * ipc // 8, (q + 1) * ipc // 8
        nc.gpsimd.dma_start(out=rb_all[:, i0:i1, :, :],
                            in_=rbm[:, i0:i1, :, :])

    # Pre-zero the 3 rotating h1 slots' j-tails so the last L2 chunk can
    # always use a full 128-col stationary (no PE tile-mode switches); the
    # zero columns produce exact-zero H2 rows for the padded j's.
    h1_slots = []
    for _ in range(3):
        t = h1pool.tile([128, 2, 512], bf16, tag="h1")
        if ctail < 128:
            nc.vector.memset(t[:, :, jpad:512], 0.0)
        h1_slots.append(t)

    xt_tiles = {}
    h1_tiles = {}
    h2_tiles = {}

    def dma_x(gd):
        # 8-row (~0.9 MB) transfers: big enough for good HBM efficiency,
        # short enough not to starve engine SBUF ports for long stretches
        t = xtpool.tile([128, 8, jpad], bf16, tag="xt")
        nc.sync.dma_start(out=t[:], in_=xr4[gd])
        xt_tiles[gd] = t
        if gd - 3 in xt_tiles:
            del xt_tiles[gd - 3]

    def l1_pair(p):
        psA = psA_t[p % 2]
        xt = xt_tiles[p // 4]
        r0 = (p % 4) * 2
        for r in range(2):
            nc.tensor.matmul(psA[:, r, 0:jpad], w1s[:], xt[:, r0 + r, :])

    def relu1(p):
        # on DVE (bias fused): h1 = max(psA + b1, 0)
        psA = psA_t[p % 2]
        t = h1pool.tile([128, 2, 512], bf16, tag="h1")
        nc.vector.tensor_scalar(
            t[:, :, 0:jpad], psA[:, 0:2, 0:jpad],
            scalar1=b1s[:], scalar2=0.0,
            op0=mybir.AluOpType.add, op1=mybir.AluOpType.max)
        h1_tiles[p] = t
        if p - 2 in h1_tiles:
            del h1_tiles[p - 2]

    def l2_pair(p):
        psB = psB_t[p % 2]
        h1t = h1_tiles[p]
        for r in range(2):
            for c in range(nch):
                nc.tensor.matmul(
                    psB[:, r, c * 128:c * 128 + 128],
                    h1t[:, r, c * 128:c * 128 + 128],
                    w2s[:],
                )

    def relu2(p):
        # on Act engine (faster per element, keeps DVE for relu1)
        psB = psB_t[p % 2]
        t = h2pool.tile([128, 2, nch, 128], bf16, tag="h2")
        nc.scalar.activation(t[:].rearrange("p a c h -> p a (c h)"),
                             psB[:, 0:2, 0:512], Relu)
        h2_tiles[p] = t
        if p - 3 in h2_tiles:
            del h2_tiles[p - 3]

    def mt_pair(p):
        # mt of iteration it=p+2 writes the tails of the psA tile of the
        # OPPOSITE parity from the one relu1(it)/L1(it) use, so the only
        # deps are >= 1 iteration stale: row i -> tile 1-(i//2)%2,
        # bank i%2, slot (i//4)%4
        h2t = h2_tiles[p]
        for r in range(2):
            i = 2 * p + r
            psA = psA_t[1 - (i // 2) % 2]
            off = jpad + ((i // 4) % 4) * 12
            for c in range(nch):
                nc.tensor.matmul(
                    psA[:, i % 2, off:off + 12],
                    h2t[:, r, c, :],
                    rb_all[:, i, c, :],
                    start=(c == 0),
                    stop=(c == nch - 1),
                )
            # stagger the two tail copies across iterations and engines so
            # neither Act nor DVE gets a double-length queue in one iter
            if i % GROUP == GROUP - 3:  # tile 1's 8 rows complete here
                g = i // GROUP
                mtc = mtpool.tile([128, 2, 48], bf16, tag="mtc1")
                nc.vector.tensor_scalar(
                    mtc[:], psA_t[1][:, 0:2, jpad:jpad + 48],
                    scalar1=0.0, scalar2=None, op0=mybir.AluOpType.add)
                nc.sync.dma_start(
                    out=mtout[g, :, 96:192],
                    in_=mtc[:].rearrange("p b s -> p (b s)"))
            if i % GROUP == GROUP - 1:  # tile 0's 8 rows complete here
                g = i // GROUP
                mtc = mtpool.tile([128, 2, 48], bf16, tag="mtc0")
                nc.scalar.activation(
                    mtc[:], psA_t[0][:, 0:2, jpad:jpad + 48], Copy)
                nc.sync.dma_start(
                    out=mtout[g, :, 0:96],
                    in_=mtc[:].rearrange("p b s -> p (b s)"))

    # pointwise MLP emitted first: it executes during the startup window
    # while the first X transfers are still in flight
    _pointwise(nc, tc, consts, psB_t, ipc,
               (pw1s, pw2s, pw3s, pb1s, pb2s, xpts), pscout)

    # software-pipelined main loop.  L1 runs one iteration AHEAD of relu1 so
    # the DVE (the saturated engine) never waits on the PE; mt trails by 2
    # iterations so its inputs are always ready.
    ngd = ipc // 8
    dma_x(0)
    dma_x(1)
    l1_pair(0)
    for it in range(npair + 2):
        if it % 4 == 0 and it // 4 + 2 < ngd:
            dma_x(it // 4 + 2)
        if it + 1 < npair:
            l1_pair(it + 1)
        if it < npair:
            relu1(it)
        if 0 <= it - 2 < npair:
            mt_pair(it - 2)
        if 0 <= it - 1 < npair:
            l2_pair(it - 1)
            relu2(it - 1)




_NC_CACHE = {}


def _get_program(jpad=JPAD_DEFAULT):
    if jpad not in _NC_CACHE:
        _NC_CACHE[jpad] = build_program(jpad)
    return _NC_CACHE[jpad]


def make_in_maps(inputs, jpad, idx_by_batch):
    """Host-side shard + preprocess for one j-chunk (idx per batch).
    Returns per-core input dicts plus aux info for the host epilogue."""
    pf = np.asarray(inputs["pointwise_features"], np.float32)
    rf = np.asarray(inputs["relative_features"], np.float32)
    rb = np.asarray(inputs["relative_basis"], np.float32)

    relb2 = np.asarray(inputs["rel_b2"], np.float32)
    assert np.all(relb2 == 0.0), (
        "kernel's flipped layer-2 assumes rel_b2 == 0 (true for this problem)"
    )

    nch = (jpad + 127) // 128

    # L1 runs in fp8 e4m3: W1 is pre-scaled by 16 so its ~0.05-magnitude
    # entries use normalized fp8 values; relu is positively homogeneous, so
    # h1 comes out scaled by 16 (bias scaled to match) and W2/16 undoes it.
    shared = {
        "w1": np.ascontiguousarray(inputs["rel_W1"], np.float32).astype(
            ml_dtypes.bfloat16),
        "w2": np.ascontiguousarray(inputs["rel_W2"], np.float32).astype(
            ml_dtypes.bfloat16),
        "b1": np.asarray(inputs["rel_b1"], np.float32).reshape(128, 1),
        "pw1": np.ascontiguousarray(inputs["pw_W1"], np.float32).astype(
            ml_dtypes.bfloat16),
        "pw2": np.ascontiguousarray(inputs["pw_W2"], np.float32).astype(
            ml_dtypes.bfloat16),
        "pw3": np.ascontiguousarray(inputs["pw_W3"], np.float32).astype(
            ml_dtypes.bfloat16),
        "pb1": np.asarray(inputs["pw_b1"], np.float32).reshape(128, 1),
        "pb2": np.asarray(inputs["pw_b2"], np.float32).reshape(128, 1),
    }

    in_maps = []
    aux = []
    for core in range(NCORES):
        b = core // (NCORES // B)
        i0 = (core % (NCORES // B)) * IPC
        sl = slice(i0, i0 + IPC)
        idx = idx_by_batch[b]
        nj = len(idx)
        assert nj <= jpad, f"unmasked atoms {nj} > jpad {jpad}"

        # X^T, j-compacted and zero-padded: [g16, f, 16, jpad]
        xc = rf[b, sl][:, idx, :]  # [IPC, nj, F]
        xcp = np.zeros((IPC, jpad, F), np.float32)
        xcp[:, :nj, :] = xc
        xr4 = np.ascontiguousarray(
            xcp.transpose(2, 0, 1)  # [F, IPC, jpad]
            .reshape(F, IPC // 8, 8, jpad)
            .transpose(1, 0, 2, 3)
        ).astype(ml_dtypes.bfloat16)

        # rbw chunks [jp, i, c, kc]; masked/padded j rows are exact zeros
        rbw = np.zeros((IPC, nch * 128, 12), np.float32)
        rbw[:, :nj, :] = rb[b, sl].reshape(IPC, N, 12)[:, idx, :]
        rbm = np.ascontiguousarray(
            rbw.reshape(IPC, nch, 128, 12).transpose(2, 0, 1, 3)
        ).astype(ml_dtypes.bfloat16)

        m = {
            "xr4": xr4,
            "rbm": rbm,
            "xpt": np.ascontiguousarray(pf[b, sl].T).astype(
                ml_dtypes.bfloat16),
        }
        m.update(shared)
        in_maps.append(m)
        aux.append({"b": b, "sl": sl, "rbsum": rbw.sum(1)})  # [IPC, 12]
    return in_maps, aux


RUN_OPTS = {}  # test harness may set e.g. {"trace": True, "tmpdir": ...}
LAST_RESULT = [None]


def kernel(**inputs):
    from concourse.bass_utils import run_bass_kernel_spmd

    me = np.asarray(inputs["masked_elements"])
    u = (~me).astype(np.float32)
    A = u.sum(-1).astype(np.float32)  # [B]
    idx_full = [np.nonzero(u[b])[0] for b in range(B)]
    nj_max = max(len(ix) for ix in idx_full)
    jpad = JPAD_DEFAULT
    # j-sum is linear: masks with more than jpad unmasked atoms are handled
    # by running the same program over j-chunks and accumulating on host
    nchunk = max(1, -(-nj_max // jpad))

    nc = _get_program(jpad)
    ngr = IPC // GROUP
    rdev_acc = [np.zeros((IPC, 12), np.float32) for _ in range(NCORES)]
    rbsum_acc = [np.zeros((IPC, 12), np.float32) for _ in range(NCORES)]
    psc_by_core = [None] * NCORES
    aux = None
    for ch in range(nchunk):
        idx_by_batch = [ix[ch * jpad:(ch + 1) * jpad] for ix in idx_full]
        in_maps, aux = make_in_maps(inputs, jpad, idx_by_batch)
        res = run_bass_kernel_spmd(nc, in_maps, core_ids=list(range(NCORES)),
                                   **RUN_OPTS)
        LAST_RESULT[0] = res
        W3rep = np.repeat(np.asarray(inputs["rel_W3"], np.float32), 3, axis=1)
        # device tail layout is (tile, bank, slot); row = 4s + 2(1-t) + b
        rowperm = np.array([4 * s + 2 * (1 - t) + bk
                            for t in range(2) for bk in range(2)
                            for s in range(4)])
        for core in range(NCORES):
            r = res.results[core]
            mtraw = np.asarray(r["mtout"], np.float32).reshape(
                ngr, 128, GROUP, 12)
            # R_dev[i, kc] = sum_h M^T[h, kc] * W3[h, k]
            summed = (mtraw * W3rep[None, :, None, :]).sum(1)  # [ngr,16,12]
            reord = np.empty_like(summed)
            reord[:, rowperm, :] = summed
            rdev_acc[core] += reord.reshape(IPC, 12)
            rbsum_acc[core] += aux[core]["rbsum"]
            psc_by_core[core] = np.asarray(r["pscout"], np.float32)

    # host epilogue: b3 term, pointwise combine, 1/A factors
    b3rep = np.repeat(np.asarray(inputs["rel_b3"], np.float32), 3)  # [12]
    pwb3 = np.asarray(inputs["pw_b3"], np.float32)  # [4]
    pb = np.asarray(inputs["pointwise_basis"], np.float32)  # [B, N, 4, 3]

    out = np.zeros((B, N, 3), np.float32)
    for core in range(NCORES):
        b, sl = aux[core]["b"], aux[core]["sl"]
        rfull = (rdev_acc[core] + b3rep[None, :] * rbsum_acc[core]) \
            / (A[b] * A[b])
        psc = psc_by_core[core].T + pwb3[None, :]  # [IPC, 4]
        pwterm = pb[b, sl].reshape(IPC, 12) * np.repeat(psc, 3, axis=1) / A[b]
        out[b, sl] = (pwterm + rfull).reshape(IPC, NB, 3).sum(1)
    return out


# revision 42
# speedup vs baseline: 1.2139x; 1.2139x over previous
"""Trainium2 Bass kernel for DenseEquivariantShiftModule.

shift[b,i,c] = ( sum_k pb[b,i,k,c]*ps[b,i,k]
               + (1/A_b) sum_k sum_j u[b,j]*rb[b,i,j,k,c]*rs[b,i,j,k] ) / A_b
where ps = MLP_pw(pointwise_features), rs = MLP_rel(relative_features),
u = ~masked, A_b = sum_j u[b,j].

Sharding: B*N = 1024 "i" rows split across 8 cores (128 rows each, each
core within one batch element).

v2 design (per core, all bf16 matmuls, f32 PSUM accumulate):
  - host compacts the j axis to the unmasked atoms (448 here) and pads to
    JPAD; masked j contribute exact zeros via zeroed rbw rows.
  - per i-row:
      L1: H1T[h,j] = w1^T @ XT         (1 matmul, 448-moving, w1 stationary)
      relu1 on Act engine, pair-batched, PSUM->SBUF bf16
      L2 flipped: H2[j,h] = (H1T chunk)^T @ w2   (4 chunk matmuls)
      relu2 on DVE (some pairs on Act), pair-batched
      mt flipped: M^T[h,kc] += (H2 chunk)^T @ rbw chunk   (4 tiny matmuls,
        12-moving, accumulated in a shared PSUM bank [128, 16 rows, 12])
  - per 16-row group the raw M^T PSUM bank is DMA'd to DRAM; the host
    epilogue applies W3 (sum over h), the b3*rbsum term, the 1/A factors,
    and combines with the pointwise path (O(N) work, like the baseline's
    host-side gather).
  - pointwise MLP runs feature-major on-device once per core
    (host pre-transposes its input), raw scales DMA'd out.
"""
import sys

sys.path.insert(0, "/opt/trn_rl_repo")

import ml_dtypes
import numpy as np

import concourse.bass as bass
import concourse.tile as tile
from concourse import mybir

B, N, F, NB = 2, 512, 128, 4
NCORES = 8
IPC = B * N // NCORES  # i-rows per core
f32 = mybir.dt.float32
bf16 = mybir.dt.bfloat16
f8 = mybir.dt.float8e4

JPAD_DEFAULT = 448  # unmasked atoms for this problem's mask (last 64 masked)
GROUP = 16  # i-rows per mt PSUM bank / output DMA group
# every RELU2_ACT_EVERY-th pair's relu2 runs on Act instead of DVE
RELU2_ACT_EVERY = 8


def _install_tile_patch():
    """walrus in this container accepts only 1 sem wait per CTRL
    instruction; TileContext's tail drain carries one per touched
    processor. Split them across SP NOPs."""
    import re

    import bass_rust
    from concourse.vector_clock import ScopedClock

    def _patched(self, tick_clock, wait_clock):
        gc = tick_clock.global_clock
        vals = eval(re.match(r"VectorClock\((\[.*\])\)", repr(gc)).group(1))
        for i, v in enumerate(vals):
            if v <= 0:
                continue
            sub = [0] * len(vals)
            sub[i] = v
            nop = self.nc.sync.nop(nofuse=True, hint="drain_wait_split")
            wait_clock.add_sem_waits(
                nop.ins, ScopedClock({None: bass_rust.VectorClock(sub)})
            )
        self.nc.sync.drain()
        self.nc.all_engine_barrier()
        assert self.sems is not None
        popped = self.nc._tile_sem_poison_stack.pop()
        assert popped is self._sem_poison
        self.nc.clear_and_free_semaphores(list(self.sems.allocated().values()))
        self.nc.all_engine_barrier()

    tile.TileContext._drain_and_barrier = _patched


def _split_multi_waits(nc):
    """This walrus build accepts a single sem wait per instruction.
    Move extra waits onto same-engine NOPs inserted just before the
    owning instruction."""
    import bass_rust

    n = 0
    for f in nc.m.functions:
        for bb in f.blocks:
            insts = bb.instructions
            i = 0
            while i < len(insts):
                ins = insts[i]
                si = ins.sync_info
                if si is not None and si.on_wait and len(si.on_wait) > 1:
                    waits = list(si.on_wait)
                    updates = list(si.on_update) if si.on_update else []
                    for w in waits[:-1]:
                        nop = mybir.InstNoOp(
                            name=f"I-waitsplit-{n}", ins=[], outs=[]
                        )
                        n += 1
                        nop.engine = ins.engine
                        nop.sync_info = bass_rust.SyncInfo(
                            on_wait=[w], on_update=[]
                        )
                        insts.insert(i, nop)
                        i += 1
                    ins.sync_info = bass_rust.SyncInfo(
                        on_wait=[waits[-1]], on_update=updates
                    )
                i += 1
    return n




def _pointwise(nc, tc, consts, psB_t, ipc, pw, pscout):
    """Pointwise MLP, feature-major (input pre-transposed on host)."""
    pw1s, pw2s, pw3s, pb1s, pb2s, xpts = pw
    Relu = mybir.ActivationFunctionType.Relu
    Copy = mybir.ActivationFunctionType.Copy
    pr = min(ipc, 512)
    nc.tensor.matmul(psB_t[0][:, 0, 0:pr], pw1s[:], xpts[:])
    h1p = consts.tile([128, ipc], bf16)
    nc.scalar.activation(h1p[:], psB_t[0][:, 0, 0:pr], Relu, bias=pb1s[:])
    nc.tensor.matmul(psB_t[0][:, 1, 0:pr], pw2s[:], h1p[:])
    h2p = consts.tile([128, ipc], bf16)
    nc.scalar.activation(h2p[:], psB_t[0][:, 1, 0:pr], Relu, bias=pb2s[:])
    nc.tensor.matmul(psB_t[1][0:NB, 0, 0:pr], pw3s[:], h2p[:])
    pscsb = consts.tile([NB, ipc], f32)
    nc.scalar.activation(pscsb[:], psB_t[1][0:NB, 0, 0:pr], Copy)
    nc.sync.dma_start(out=pscout[:], in_=pscsb[:])

def build_program(jpad=JPAD_DEFAULT, ipc=IPC, split_waits=True):
    _install_tile_patch()
    assert jpad % 64 == 0
    nch = (jpad + 127) // 128  # j chunks (last may be partial)
    ng4 = ipc // 4  # 4-row DMA groups
    ngr = ipc // GROUP  # output groups
    nc = bass.Bass()

    xr4 = nc.dram_tensor("xr4", [ipc // 8, F, 8, jpad], bf16,
                         kind="ExternalInput")
    rbm = nc.dram_tensor("rbm", [128, ipc, nch, 12], bf16, kind="ExternalInput")
    w1 = nc.dram_tensor("w1", [F, 128], bf16, kind="ExternalInput")
    w2 = nc.dram_tensor("w2", [128, 128], bf16, kind="ExternalInput")
    b1 = nc.dram_tensor("b1", [128, 1], f32, kind="ExternalInput")
    xpt = nc.dram_tensor("xpt", [F, ipc], bf16, kind="ExternalInput")
    pw1 = nc.dram_tensor("pw1", [F, 128], bf16, kind="ExternalInput")
    pw2 = nc.dram_tensor("pw2", [128, 128], bf16, kind="ExternalInput")
    pw3 = nc.dram_tensor("pw3", [128, NB], bf16, kind="ExternalInput")
    pb1 = nc.dram_tensor("pb1", [128, 1], f32, kind="ExternalInput")
    pb2 = nc.dram_tensor("pb2", [128, 1], f32, kind="ExternalInput")
    mtout = nc.dram_tensor("mtout", [ngr, 128, GROUP * 12], bf16,
                           kind="ExternalOutput")
    pscout = nc.dram_tensor("pscout", [NB, ipc], f32, kind="ExternalOutput")

    from contextlib import ExitStack

    with tile.TileContext(nc) as tc:
        with ExitStack() as ctx:
            _kernel_body(ctx, tc, jpad, nch, ipc,
                         xr4, rbm, (w1, w2, b1),
                         (xpt, pw1, pw2, pw3, pb1, pb2),
                         mtout, pscout)
    if split_waits:
        _split_multi_waits(nc)
    return nc


def _kernel_body(ctx, tc, jpad, nch, ipc, xr4, rbm, relw, pww, mtout, pscout):
    nc = tc.nc
    w1, w2, b1 = relw
    xpt, pw1, pw2, pw3, pb1, pb2 = pww
    Relu = mybir.ActivationFunctionType.Relu
    npair = ipc // 2
    ng4 = ipc // 4
    ngr = ipc // GROUP
    ctail = jpad - (nch - 1) * 128  # width of last j-chunk (64 or 128)

    Copy = mybir.ActivationFunctionType.Copy
    consts = ctx.enter_context(tc.tile_pool(name="consts", bufs=1))
    xtpool = ctx.enter_context(tc.tile_pool(name="xt", bufs=2))
    h1pool = ctx.enter_context(tc.tile_pool(name="h1", bufs=3))
    h2pool = ctx.enter_context(tc.tile_pool(name="h2", bufs=3))
    mtpool = ctx.enter_context(tc.tile_pool(name="mtc", bufs=2))
    ps_a = ctx.enter_context(tc.tile_pool(name="ps_a", bufs=1, space="PSUM"))
    ps_b = ctx.enter_context(tc.tile_pool(name="ps_b", bufs=1, space="PSUM"))

    # Static PSUM: 4 banks for L1 outs, 4 banks for L2 outs, organized as
    # TWO tiles per stage (one per pair parity).  Dependency tracking is
    # effectively whole-tile for cross-engine hazards, so separate tiles —
    # not just separate banks — are required for consecutive pairs to
    # pipeline without WAR serialization.
    # The mt accumulators live in the unused 64-f32 tails of the psA banks
    # (L1 writes only [0:jpad]); mt of pair p writes the OPPOSITE parity's
    # tile from the one relu1(p) is reading, so they never couple.
    assert jpad + 4 * 12 <= 512, "mt tails must fit beside L1 outputs"
    psA_t = [ps_a.tile([128, 2, 512], f32, tag=f"a{t}", name=f"psA{t}")
             for t in range(2)]
    psB_t = [ps_b.tile([128, 2, 512], f32, tag=f"b{t}", name=f"psB{t}")
             for t in range(2)]

    # constants (tiny, ahead of the first big X transfer on the same queue)
    w1s = consts.tile([128, 128], bf16)
    nc.sync.dma_start(out=w1s[:], in_=w1[:])
    w2s = consts.tile([128, 128], bf16)
    nc.sync.dma_start(out=w2s[:], in_=w2[:])
    b1s = consts.tile([128, 1], f32)
    nc.sync.dma_start(out=b1s[:], in_=b1[:])

    # pointwise constants first on the SWDGE queue (tiny): the pointwise
    # MLP runs during the startup window while the first X transfer lands
    pw1s = consts.tile([128, 128], bf16)
    nc.gpsimd.dma_start(out=pw1s[:], in_=pw1[:])
    pw2s = consts.tile([128, 128], bf16)
    nc.gpsimd.dma_start(out=pw2s[:], in_=pw2[:])
    pw3s = consts.tile([128, NB], bf16)
    nc.gpsimd.dma_start(out=pw3s[:], in_=pw3[:])
    pb1s = consts.tile([128, 1], f32)
    nc.gpsimd.dma_start(out=pb1s[:], in_=pb1[:])
    pb2s = consts.tile([128, 1], f32)
    nc.gpsimd.dma_start(out=pb2s[:], in_=pb2[:])
    xpts = consts.tile([128, ipc], bf16)
    nc.gpsimd.dma_start(out=xpts[:], in_=xpt[:])

    # rbw chunks, resident in SBUF: [jp, i, c, kc]; SWDGE path (gpsimd)
    # so they stream in parallel with the first X transfers
    rb_all = consts.tile([128, ipc, nch, 12], bf16)
    for q in range(8):
        i0, i1 = q * ipc // 8, (q + 1) * ipc // 8
        nc.gpsimd.dma_start(out=rb_all[:, i0:i1, :, :],
                            in_=rbm[:, i0:i1, :, :])

    # Pre-zero the 3 rotating h1 slots' j-tails so the last L2 chunk can
    # always use a full 128-col stationary (no PE tile-mode switches); the
    # zero columns produce exact-zero H2 rows for the padded j's.
    h1_slots = []
    for _ in range(3):
        t = h1pool.tile([128, 2, 512], bf16, tag="h1")
        if ctail < 128:
            nc.vector.memset(t[:, :, jpad:512], 0.0)
        h1_slots.append(t)

    xt_tiles = {}
    h1_tiles = {}
    h2_tiles = {}

    def dma_x(gd):
        # 8-row (~0.9 MB) transfers: big enough for good HBM efficiency,
        # short enough not to starve engine SBUF ports for long stretches
        t = xtpool.tile([128, 8, jpad], bf16, tag="xt")
        nc.sync.dma_start(out=t[:], in_=xr4[gd])
        xt_tiles[gd] = t
        if gd - 3 in xt_tiles:
            del xt_tiles[gd - 3]

    def l1_pair(p):
        psA = psA_t[p % 2]
        xt = xt_tiles[p // 4]
        r0 = (p % 4) * 2
        for r in range(2):
            nc.tensor.matmul(psA[:, r, 0:jpad], w1s[:], xt[:, r0 + r, :])

    def relu1(p):
        # on DVE (bias fused): h1 = max(psA + b1, 0)
        psA = psA_t[p % 2]
        t = h1pool.tile([128, 2, 512], bf16, tag="h1")
        nc.vector.tensor_scalar(
            t[:, :, 0:jpad], psA[:, 0:2, 0:jpad],
            scalar1=b1s[:], scalar2=0.0,
            op0=mybir.AluOpType.add, op1=mybir.AluOpType.max)
        h1_tiles[p] = t
        if p - 2 in h1_tiles:
            del h1_tiles[p - 2]

    def l2_pair(p):
        psB = psB_t[p % 2]
        h1t = h1_tiles[p]
        for r in range(2):
            for c in range(nch):
                nc.tensor.matmul(
                    psB[:, r, c * 128:c * 128 + 128],
                    h1t[:, r, c * 128:c * 128 + 128],
                    w2s[:],
                )

    def relu2(p):
        # on Act engine (faster per element, keeps DVE for relu1)
        psB = psB_t[p % 2]
        t = h2pool.tile([128, 2, nch, 128], bf16, tag="h2")
        nc.scalar.activation(t[:].rearrange("p a c h -> p a (c h)"),
                             psB[:, 0:2, 0:512], Relu)
        h2_tiles[p] = t
        if p - 3 in h2_tiles:
            del h2_tiles[p - 3]

    def mt_pair(p):
        # mt of iteration it=p+2 writes the tails of the psA tile of the
        # OPPOSITE parity from the one relu1(it)/L1(it) use, so the only
        # deps are >= 1 iteration stale: row i -> tile 1-(i//2)%2,
        # bank i%2, slot (i//4)%4
        h2t = h2_tiles[p]
        for r in range(2):
            i = 2 * p + r
            psA = psA_t[1 - (i // 2) % 2]
            off = jpad + ((i // 4) % 4) * 12
            for c in range(nch):
                nc.tensor.matmul(
                    psA[:, i % 2, off:off + 12],
                    h2t[:, r, c, :],
                    rb_all[:, i, c, :],
                    start=(c == 0),
                    stop=(c == nch - 1),
                )
            # stagger the two tail copies across iterations and engines so
            # neither Act nor DVE gets a double-length queue in one iter
            if i % GROUP == GROUP - 3:  # tile 1's 8 rows complete here
                g = i // GROUP
                mtc = mtpool.tile([128, 2, 48], bf16, tag="mtc1")
                nc.vector.tensor_scalar(
                    mtc[:], psA_t[1][:, 0:2, jpad:jpad + 48],
                    scalar1=0.0, scalar2=None, op0=mybir.AluOpType.add)
                nc.sync.dma_start(
                    out=mtout[g, :, 96:192],
                    in_=mtc[:].rearrange("p b s -> p (b s)"))
            if i % GROUP == GROUP - 1:  # tile 0's 8 rows complete here
                g = i // GROUP
                mtc = mtpool.tile([128, 2, 48], bf16, tag="mtc0")
                nc.scalar.activation(
                    mtc[:], psA_t[0][:, 0:2, jpad:jpad + 48], Copy)
                nc.sync.dma_start(
                    out=mtout[g, :, 0:96],
                    in_=mtc[:].rearrange("p b s -> p (b s)"))

    # pointwise MLP emitted first: it executes during the startup window
    # while the first X transfers are still in flight
    _pointwise(nc, tc, consts, psB_t, ipc,
               (pw1s, pw2s, pw3s, pb1s, pb2s, xpts), pscout)

    # software-pipelined main loop.  L1 runs one iteration AHEAD of relu1 so
    # the DVE (the saturated engine) never waits on the PE; mt trails by 2
    # iterations so its inputs are always ready.
    ngd = ipc // 8
    dma_x(0)
    dma_x(1)
    for it in range(npair + 2):
        if it % 4 == 0 and it // 4 + 2 < ngd:
            dma_x(it // 4 + 2)
        if it < npair:
            l1_pair(it)
            relu1(it)
        if 0 <= it - 2 < npair:
            mt_pair(it - 2)
        if 0 <= it - 1 < npair:
            l2_pair(it - 1)
            relu2(it - 1)




_NC_CACHE = {}


def _get_program(jpad=JPAD_DEFAULT):
    if jpad not in _NC_CACHE:
        _NC_CACHE[jpad] = build_program(jpad)
    return _NC_CACHE[jpad]


def make_in_maps(inputs, jpad, idx_by_batch):
    """Host-side shard + preprocess for one j-chunk (idx per batch).
    Returns per-core input dicts plus aux info for the host epilogue."""
    pf = np.asarray(inputs["pointwise_features"], np.float32)
    rf = np.asarray(inputs["relative_features"], np.float32)
    rb = np.asarray(inputs["relative_basis"], np.float32)

    relb2 = np.asarray(inputs["rel_b2"], np.float32)
    assert np.all(relb2 == 0.0), (
        "kernel's flipped layer-2 assumes rel_b2 == 0 (true for this problem)"
    )

    nch = (jpad + 127) // 128

    # L1 runs in fp8 e4m3: W1 is pre-scaled by 16 so its ~0.05-magnitude
    # entries use normalized fp8 values; relu is positively homogeneous, so
    # h1 comes out scaled by 16 (bias scaled to match) and W2/16 undoes it.
    shared = {
        "w1": np.ascontiguousarray(inputs["rel_W1"], np.float32).astype(
            ml_dtypes.bfloat16),
        "w2": np.ascontiguousarray(inputs["rel_W2"], np.float32).astype(
            ml_dtypes.bfloat16),
        "b1": np.asarray(inputs["rel_b1"], np.float32).reshape(128, 1),
        "pw1": np.ascontiguousarray(inputs["pw_W1"], np.float32).astype(
            ml_dtypes.bfloat16),
        "pw2": np.ascontiguousarray(inputs["pw_W2"], np.float32).astype(
            ml_dtypes.bfloat16),
        "pw3": np.ascontiguousarray(inputs["pw_W3"], np.float32).astype(
            ml_dtypes.bfloat16),
        "pb1": np.asarray(inputs["pw_b1"], np.float32).reshape(128, 1),
        "pb2": np.asarray(inputs["pw_b2"], np.float32).reshape(128, 1),
    }

    in_maps = []
    aux = []
    for core in range(NCORES):
        b = core // (NCORES // B)
        i0 = (core % (NCORES // B)) * IPC
        sl = slice(i0, i0 + IPC)
        idx = idx_by_batch[b]
        nj = len(idx)
        assert nj <= jpad, f"unmasked atoms {nj} > jpad {jpad}"

        # X^T, j-compacted and zero-padded: [g16, f, 16, jpad]
        xc = rf[b, sl][:, idx, :]  # [IPC, nj, F]
        xcp = np.zeros((IPC, jpad, F), np.float32)
        xcp[:, :nj, :] = xc
        xr4 = np.ascontiguousarray(
            xcp.transpose(2, 0, 1)  # [F, IPC, jpad]
            .reshape(F, IPC // 8, 8, jpad)
            .transpose(1, 0, 2, 3)
        ).astype(ml_dtypes.bfloat16)

        # rbw chunks [jp, i, c, kc]; masked/padded j rows are exact zeros
        rbw = np.zeros((IPC, nch * 128, 12), np.float32)
        rbw[:, :nj, :] = rb[b, sl].reshape(IPC, N, 12)[:, idx, :]
        rbm = np.ascontiguousarray(
            rbw.reshape(IPC, nch, 128, 12).transpose(2, 0, 1, 3)
        ).astype(ml_dtypes.bfloat16)

        m = {
            "xr4": xr4,
            "rbm": rbm,
            "xpt": np.ascontiguousarray(pf[b, sl].T).astype(
                ml_dtypes.bfloat16),
        }
        m.update(shared)
        in_maps.append(m)
        aux.append({"b": b, "sl": sl, "rbsum": rbw.sum(1)})  # [IPC, 12]
    return in_maps, aux


RUN_OPTS = {}  # test harness may set e.g. {"trace": True, "tmpdir": ...}
LAST_RESULT = [None]


def kernel(**inputs):
    from concourse.bass_utils import run_bass_kernel_spmd

    me = np.asarray(inputs["masked_elements"])
    u = (~me).astype(np.float32)
    A = u.sum(-1).astype(np.float32)  # [B]
    idx_full = [np.nonzero(u[b])[0] for b in range(B)]
    nj_max = max(len(ix) for ix in idx_full)
    jpad = JPAD_DEFAULT
    # j-sum is linear: masks with more than jpad unmasked atoms are handled
    # by running the same program over j-chunks and accumulating on host
    nchunk = max(1, -(-nj_max // jpad))

    nc = _get_program(jpad)
    ngr = IPC // GROUP
    rdev_acc = [np.zeros((IPC, 12), np.float32) for _ in range(NCORES)]
    rbsum_acc = [np.zeros((IPC, 12), np.float32) for _ in range(NCORES)]
    psc_by_core = [None] * NCORES
    aux = None
    for ch in range(nchunk):
        idx_by_batch = [ix[ch * jpad:(ch + 1) * jpad] for ix in idx_full]
        in_maps, aux = make_in_maps(inputs, jpad, idx_by_batch)
        res = run_bass_kernel_spmd(nc, in_maps, core_ids=list(range(NCORES)),
                                   **RUN_OPTS)
        LAST_RESULT[0] = res
        W3rep = np.repeat(np.asarray(inputs["rel_W3"], np.float32), 3, axis=1)
        # device tail layout is (tile, bank, slot); row = 4s + 2(1-t) + b
        rowperm = np.array([4 * s + 2 * (1 - t) + bk
                            for t in range(2) for bk in range(2)
                            for s in range(4)])
        for core in range(NCORES):
            r = res.results[core]
            mtraw = np.asarray(r["mtout"], np.float32).reshape(
                ngr, 128, GROUP, 12)
            # R_dev[i, kc] = sum_h M^T[h, kc] * W3[h, k]
            summed = (mtraw * W3rep[None, :, None, :]).sum(1)  # [ngr,16,12]
            reord = np.empty_like(summed)
            reord[:, rowperm, :] = summed
            rdev_acc[core] += reord.reshape(IPC, 12)
            rbsum_acc[core] += aux[core]["rbsum"]
            psc_by_core[core] = np.asarray(r["pscout"], np.float32)

    # host epilogue: b3 term, pointwise combine, 1/A factors
    b3rep = np.repeat(np.asarray(inputs["rel_b3"], np.float32), 3)  # [12]
    pwb3 = np.asarray(inputs["pw_b3"], np.float32)  # [4]
    pb = np.asarray(inputs["pointwise_basis"], np.float32)  # [B, N, 4, 3]

    out = np.zeros((B, N, 3), np.float32)
    for core in range(NCORES):
        b, sl = aux[core]["b"], aux[core]["sl"]
        rfull = (rdev_acc[core] + b3rep[None, :] * rbsum_acc[core]) \
            / (A[b] * A[b])
        psc = psc_by_core[core].T + pwb3[None, :]  # [IPC, 4]
        pwterm = pb[b, sl].reshape(IPC, 12) * np.repeat(psc, 3, axis=1) / A[b]
        out[b, sl] = (pwterm + rfull).reshape(IPC, NB, 3).sum(1)
    return out
